# revision 4
# baseline (speedup 1.0000x reference)
"""Causal multi-head attention block (qkv proj + causal softmax attention + out proj)
for Trainium2, sharded over 8 NeuronCores: data-parallel over batch (2) x
tensor-parallel over heads (4 heads per core of 16).

Schedule: chunk-software-pipelined. The attention inner loop (scores ->
exp -> attnT) is ACT(exp)-rate-limited, so the qkv projection of chunk
qc+1 and the out-projection of chunk qc-1 are *injected* between
attention steps via a feeder, keeping the in-order PE queue busy during
exp waits. Exps are paired: two consecutive score tiles of one head are
written offset-packed into one 2-bank PSUM tile and activated with a
single wide ACTIVATE, halving ACT instruction overhead.

Per core (batch b, 4 heads):
  qT,kT [hd, S] and v [S, hd]    (qkv projection, weights pre-transposed on host)
  ST    [k, q] = transposed scores, causal-blocked
  P     = exp(ST + mask)
  attnT [hd+1, q] = [v | 1].T @ P   (row hd = softmax denominator)
  attn_n = attnT / denom
  out_partial [S, D] = attn_n.T @ owT  (row-parallel out proj)
Host sums the 4 per-core partials of each batch.
"""

import os
import sys

import numpy as np

sys.path.insert(0, "/opt/trn_rl_repo")

import concourse.bass as bass
import concourse.tile as tile
from concourse import bacc, mybir
from concourse.bass import MemorySpace
from concourse.bass_utils import run_bass_kernel_spmd

F32 = mybir.dt.float32
EXP = mybir.ActivationFunctionType.Exp
BF16 = mybir.dt.bfloat16

B, S, D = 2, 2048, 1024
H, HD = 16, 64
NCORES = 8
NH = 4          # heads per core
NP = 2          # head pairs per core
SCALE = HD ** -0.5

N_DT = D // 128          # 8 d-tiles of 128
N_ST = S // 128          # 16 seq tiles of 128
N_CH = S // 512          # 4 seq chunks of 512
FQK = 2 * NH * HD // 128  # 4 f-tiles covering q|k (pair-major)
VW = NH * HD             # 256 v columns
NEG = -1.0e9

MM_DT = BF16
DEPTH = 1   # attnT pipeline depth, in kt-PAIR units (= 2 kt of slack)


def _emit(tc, nc, xT_d, wT_d, owT_d, mask_d, out_d):
    import contextlib

    ctx = contextlib.ExitStack()
    with ctx:
        # ---------------- pools ----------------
        sb = ctx.enter_context(tc.tile_pool(name="sb", bufs=1))
        p_pool = ctx.enter_context(tc.tile_pool(name="psb", bufs=6))
        an_pool = ctx.enter_context(tc.tile_pool(name="attn_n", bufs=4))
        sm_pool = ctx.enter_context(tc.tile_pool(name="smalls", bufs=8))
        out_pool = ctx.enter_context(tc.tile_pool(name="outsb", bufs=4))
        # PSUM: 8 banks of [128 x 2KB]:
        #   ps_st : 2 x [128,1024] f32 (paired score tiles)      -> 4 banks
        #   ps_at : 2 x [65->128,512] f32 (attnT accumulators)   -> 2 banks
        #   ps_acc: 2 x [128,512] f32 (qkv + outproj chains)     -> 2 banks
        ps_acc = ctx.enter_context(
            tc.tile_pool(name="ps_acc", bufs=2, space=MemorySpace.PSUM))
        ps_at = ctx.enter_context(
            tc.tile_pool(name="ps_at", bufs=2, space=MemorySpace.PSUM))
        ps_st = ctx.enter_context(
            tc.tile_pool(name="ps_st", bufs=2, space=MemorySpace.PSUM))

        # ---------------- static SBUF tiles ----------------
        # qT/kT: tile h in [0,4) = qT of head h in rows 0:64 (rows 64:128
        # zeroed so scores matmuls run K=128 full-row); tile 4+h = kT.
        qk_sb = [sb.tile([128, S], MM_DT, tag=f"qk{i}", name=f"qk{i}")
                 for i in range(2 * NH)]
        # v: per seq-tile [128, 4 heads, 65] (64 v cols + ones col)
        v_sb = [sb.tile([128, NH, HD + 1], MM_DT, tag=f"v{i}", name=f"v{i}")
                for i in range(N_ST)]
        mask_sb = sb.tile([128, 640], F32)
        owT_sb = [sb.tile([128, D], MM_DT, tag=f"ow{i}", name=f"ow{i}")
                  for i in range(NP)]
        xT_sb = [sb.tile([128, S], MM_DT, tag=f"x{i}", name=f"x{i}")
                 for i in range(N_DT)]
        wT_sb = [sb.tile([128, 3 * VW], MM_DT, tag=f"w{i}", name=f"w{i}")
                 for i in range(N_DT)]
        warm_sb = sb.tile([128, 512], MM_DT)

        # init memsets run on gpsimd/vector during the input DMAs
        nc.vector.memset(warm_sb, 0.0)
        for t in qk_sb:
            nc.gpsimd.memset(t[HD:128, :], 0.0)
        for t in v_sb:
            nc.gpsimd.memset(t, 1.0)   # ones col pre-set; data cols overwritten

        # ---------------- input DMAs (priority order) ----------------
        # first qkv chain needs wT[d] + xT[d] cols 0:512 -> interleave those.
        for d in range(N_DT):
            nc.sync.dma_start(out=wT_sb[d], in_=wT_d[d * 128:(d + 1) * 128, :])
            nc.sync.dma_start(out=xT_sb[d][:, 0:512],
                              in_=xT_d[d * 128:(d + 1) * 128, 0:512])
        nc.sync.dma_start(out=mask_sb, in_=mask_d)
        for sch in range(1, N_CH):
            for d in range(N_DT):
                nc.sync.dma_start(
                    out=xT_sb[d][:, sch * 512:(sch + 1) * 512],
                    in_=xT_d[d * 128:(d + 1) * 128, sch * 512:(sch + 1) * 512])
        for p in range(NP):
            nc.sync.dma_start(out=owT_sb[p], in_=owT_d[p * 128:(p + 1) * 128, :])

        # HAM warm-up: dependency-free matmuls ramp the PE clock while the
        # first input DMAs stream.
        wu_ps = ps_st.tile([128, 1024], F32, tag="st", name="wu_ps")
        for _ in range(12):
            nc.tensor.matmul(wu_ps[:, 0:512], warm_sb[:, 0:128], warm_sb,
                             start=True, stop=True)

        # ---------------- feeder item generators ----------------
        def chain_qk(f, sch):
            """qT/kT projection chain: out[f 128, s 512] += wT.T @ xT."""
            pss = ps_acc.tile([128, 512], F32, tag="acc", name="psqk")
            for d in range(N_DT):
                yield lambda d=d, pss=pss: nc.tensor.matmul(
                    pss,
                    wT_sb[d][:, f * 128:(f + 1) * 128],
                    xT_sb[d][:, sch * 512:(sch + 1) * 512],
                    start=(d == 0),
                    stop=(d == N_DT - 1),
                )

            def cp(pss=pss):
                # split the two 64-row halves across ACT and DVE
                nc.scalar.copy(
                    qk_sb[2 * f][0:HD, sch * 512:(sch + 1) * 512],
                    pss[0:HD, :])
                nc.vector.tensor_copy(
                    qk_sb[2 * f + 1][0:HD, sch * 512:(sch + 1) * 512],
                    pss[HD:128, :])
            yield cp

        def chain_v(st):
            """v projection chain for one seq tile: [s 128, 256]."""
            psv = ps_acc.tile([128, VW], F32, tag="acc", name="psv")
            for d in range(N_DT):
                yield lambda d=d, psv=psv: nc.tensor.matmul(
                    psv,
                    xT_sb[d][:, st * 128:(st + 1) * 128],
                    wT_sb[d][:, 2 * VW:3 * VW],
                    start=(d == 0),
                    stop=(d == N_DT - 1),
                )
            yield lambda psv=psv: nc.vector.tensor_copy(
                v_sb[st][:, :, 0:HD],
                psv.rearrange("p (h d) -> p h d", h=NH),
            )

        an_hist = {}
        pend_norm = {}

        def outproj_units(qc):
            """out_partial rows qc*512:(qc+1)*512 = attn_n.T @ owT."""
            an = an_hist.pop(qc)
            k = 0
            for qs in range(4):
                qsl = slice(qs * 128, (qs + 1) * 128)
                for e in range(2):
                    ops = ps_acc.tile([128, 512], F32, tag="acc", name="psout")
                    for p in range(NP):
                        yield lambda ops=ops, p=p, an=an, qsl=qsl, e=e: \
                            nc.tensor.matmul(
                                ops,
                                an[p][:, qsl],
                                owT_sb[p][:, e * 512:(e + 1) * 512],
                                start=(p == 0),
                                stop=(p == NP - 1),
                            )

                    def cpdma(ops=ops, qs=qs, e=e, k=k):
                        osb = out_pool.tile([128, 512], F32, tag="osb",
                                            name="osb")
                        if k % 2 == 0:
                            nc.vector.tensor_copy(osb, ops)
                        else:
                            nc.scalar.copy(osb, ops)
                        nc.sync.dma_start(
                            out=out_d[qc * 512 + qs * 128:
                                      qc * 512 + (qs + 1) * 128,
                                      e * 512:(e + 1) * 512],
                            in_=osb,
                        )
                    yield cpdma
                    k += 1

        class Feeder:
            def __init__(self, gens):
                self.items = []
                for g in gens:
                    self.items.append(g)
                self.emitted = 0
                self.cur = None
                self.total = 0

            def pump_frac(self, frac):
                tgt = int(self.total * frac + 0.999)
                while self.emitted < tgt:
                    if self.cur is None:
                        if not self.items:
                            return
                        self.cur = self.items.pop(0)
                    try:
                        thunk = next(self.cur)
                    except StopIteration:
                        self.cur = None
                        continue
                    thunk()
                    self.emitted += 1

        # ---------------- attention chunk ----------------
        def emit_attention(qc, feeder):
            n_kt = 4 * (qc + 1)
            n_pairs = n_kt // 2
            steps_per_p = n_pairs + DEPTH
            tot_steps = NP * steps_per_p
            step = 0
            for p in range(NP):
                at_ps = [ps_at.tile([HD + 1, 512], F32, tag="at", name="at_ps")
                         for _ in range(2)]
                pend = {}
                for m in range(steps_per_p):
                    if m < n_pairs:
                        kt0, kt1 = 2 * m, 2 * m + 1
                        j0, j1 = kt0 - 4 * qc, kt1 - 4 * qc
                        rs0 = 0 if j0 < 0 else j0 * 128
                        rs1 = 0 if j1 < 0 else j1 * 128
                        n0, n1 = 512 - rs0, 512 - rs1
                        for hp in range(2):
                            h = 2 * p + hp
                            st_t = ps_st.tile([128, 1024], F32, tag="st",
                                              name="st_t")
                            # kt0 scores at cols [0:n0]
                            nc.tensor.matmul(
                                st_t[:, 0:n0],
                                qk_sb[NH + h][:, kt0 * 128:(kt0 + 1) * 128],
                                qk_sb[h][:, qc * 512 + rs0:(qc + 1) * 512],
                                start=True, stop=True,
                            )
                            if j0 >= 0:
                                nc.vector.tensor_add(
                                    st_t[:, 0:128], st_t[:, 0:128],
                                    mask_sb[:, 128:256])
                            # kt1 scores packed at cols [n0:n0+n1]
                            nc.tensor.matmul(
                                st_t[:, n0:n0 + n1],
                                qk_sb[NH + h][:, kt1 * 128:(kt1 + 1) * 128],
                                qk_sb[h][:, qc * 512 + rs1:(qc + 1) * 512],
                                start=True, stop=True,
                            )
                            if j1 >= 0:
                                nc.vector.tensor_add(
                                    st_t[:, n0:n0 + 128], st_t[:, n0:n0 + 128],
                                    mask_sb[:, 128:256])
                            # one wide exp over both tiles
                            p_t = p_pool.tile([128, 1024], MM_DT, tag="p",
                                              name="p_t")
                            nc.scalar.activation(
                                p_t[:, 0:n0 + n1], st_t[:, 0:n0 + n1], EXP)
                            pend[(m, hp)] = (p_t, rs0, n0, rs1, n1)
                    if m >= DEPTH:
                        md = m - DEPTH
                        for hp in range(2):
                            p_t, rs0, n0, rs1, n1 = pend.pop((md, hp))
                            kt0, kt1 = 2 * md, 2 * md + 1
                            nc.tensor.matmul(
                                at_ps[hp][:, rs0:512],
                                v_sb[kt0][:, 2 * p + hp, :],
                                p_t[:, 0:n0],
                                start=(md == 0),
                                stop=False,
                            )
                            nc.tensor.matmul(
                                at_ps[hp][:, rs1:512],
                                v_sb[kt1][:, 2 * p + hp, :],
                                p_t[:, n0:n0 + n1],
                                start=False,
                                stop=(md == n_pairs - 1),
                            )
                    step += 1
                    feeder.pump_frac(step / tot_steps)
                # release the accumulator banks fast (two DVE copies);
                # the reciprocal chain is deferred to emit_normalize.
                for hp in range(2):
                    anu = an_pool.tile([HD, 512], MM_DT, tag="anu", name="anu")
                    nc.vector.tensor_copy(anu, at_ps[hp][0:HD, :])
                    lsb = sm_pool.tile([1, 512], F32, tag="lsb", name="lsb")
                    nc.vector.tensor_copy(lsb, at_ps[hp][HD:HD + 1, :])
                    pend_norm[(qc, p, hp)] = (anu, lsb)
            feeder.pump_frac(1.0)

        def norm_gen(qc):
            """Normalize attnT by the softmax denominator.

            Yielded as feeder items with bcast(i+1) emitted before mul(i) so
            a DVE mul never head-blocks waiting on the gpsimd broadcast."""
            an = [an_pool.tile([128, 512], MM_DT, tag=f"an{p}", name=f"an{p}",
                               bufs=4)
                  for p in range(NP)]
            an_hist[qc] = an
            units = [(p, hp) for p in range(NP) for hp in range(2)]
            recs = []

            def recs_thunk():
                for p, hp in units:
                    anu, lsb = pend_norm.pop((qc, p, hp))
                    rec = sm_pool.tile([1, 512], F32, tag="rec", name="rec",
                                       bufs=2)
                    nc.vector.reciprocal_approx_fast(rec, lsb)
                    rec16 = sm_pool.tile([1, 512], MM_DT, tag="rec16",
                                         name="rec16", bufs=8)
                    nc.vector.tensor_copy(rec16, rec)
                    recs.append((p, hp, anu, rec16))
            yield recs_thunk

            bcs = []

            def bc_thunk(i):
                p, hp, anu, rec16 = recs[i]
                bc = sm_pool.tile([HD, 512], MM_DT, tag="bc", name="bc",
                                  bufs=4)
                nc.gpsimd.partition_broadcast(bc, rec16)
                bcs.append(bc)

            def mul_thunk(i):
                p, hp, anu, rec16 = recs[i]
                nc.vector.tensor_mul(
                    an[p][hp * HD:(hp + 1) * HD, :], anu, bcs[i])

            yield lambda: bc_thunk(0)
            yield lambda: bc_thunk(1)
            yield lambda: mul_thunk(0)
            yield lambda: bc_thunk(2)
            yield lambda: mul_thunk(1)
            yield lambda: bc_thunk(3)
            yield lambda: mul_thunk(2)
            yield lambda: mul_thunk(3)

        # ---------------- schedule ----------------
        # pre-loop: qk projection of chunk 0 (v(0..3) arrives via feeder)
        for it in chain_qk(0, 0):
            it()
        for it in chain_qk(1, 0):
            it()
        for it in chain_qk(2, 0):
            it()
        for it in chain_qk(3, 0):
            it()

        for qc in range(N_CH):
            gens = []
            if qc == 0:
                gens += [chain_v(st) for st in range(4)]
                gens += [chain_qk(f, 1) for f in range(FQK)]
                gens += [chain_v(st) for st in range(4, 8)]
                total = 9 * 12
            elif qc < N_CH - 1:
                gens += [norm_gen(qc - 1)]
                gens += [chain_qk(f, qc + 1) for f in range(FQK)]
                gens += [chain_v(st) for st in range(4 * qc + 4, 4 * qc + 8)]
                gens += [outproj_units(qc - 1)]
                total = 9 + 9 * 8 + 32
            else:
                gens = [norm_gen(qc - 1), outproj_units(qc - 1)]
                total = 9 + 32
            feeder = Feeder(gens)
            feeder.total = total
            emit_attention(qc, feeder)

        for it in norm_gen(N_CH - 1):
            it()
        for it in outproj_units(N_CH - 1):
            it()


_CACHE = {}


def _build():
    if "nc" in _CACHE:
        return _CACHE["nc"]
    nc = bacc.Bacc("TRN2", target_bir_lowering=False, debug=False)
    xT_d = nc.dram_tensor("xT", [D, S], MM_DT, kind="ExternalInput").ap()
    wT_d = nc.dram_tensor("wT", [D, 3 * VW], MM_DT, kind="ExternalInput").ap()
    owT_d = nc.dram_tensor("owT", [VW, D], MM_DT, kind="ExternalInput").ap()
    mask_d = nc.dram_tensor("mask", [128, 640], F32, kind="ExternalInput").ap()
    out_d = nc.dram_tensor("out", [S, D], F32, kind="ExternalOutput").ap()
    with tile.TileContext(nc) as tc:
        _emit(tc, nc, xT_d, wT_d, owT_d, mask_d, out_d)
    nc.compile()
    _CACHE["nc"] = nc
    return nc


def _mask_np():
    # [128, 640]: cols 0-127 all NEG, cols 128-255 lower-triangular keep
    # (col >= row -> 0 else NEG), cols 256-639 zeros.
    m = np.zeros((128, 640), np.float32)
    m[:, 0:128] = NEG
    r = np.arange(128)
    tri = np.where(r[None, :] >= r[:, None], 0.0, NEG).astype(np.float32)
    m[:, 128:256] = tri
    return m


def make_in_maps(x, qkv_w, out_w):
    """Per-core input dicts for the 8-way (batch x head-group) sharding."""
    x = np.asarray(x, np.float32)
    qkv_w = np.asarray(qkv_w, np.float32)
    out_w = np.asarray(out_w, np.float32)
    xT = [np.ascontiguousarray(x[b].T) for b in range(B)]
    mask = _mask_np()
    import ml_dtypes
    np_mm = ml_dtypes.bfloat16
    in_maps = []
    for c in range(NCORES):
        b = c // 4
        h0 = (c % 4) * NH
        rows = np.arange(h0 * HD, (h0 + NH) * HD)
        wq = qkv_w[rows] * np.float32(SCALE)
        wk = qkv_w[D + rows]
        wv = qkv_w[2 * D + rows]
        wT = np.ascontiguousarray(np.concatenate([wq, wk, wv], 0).T)
        owT = np.ascontiguousarray(out_w[:, rows].T)
        in_maps.append({"xT": xT[b].astype(np_mm), "wT": wT.astype(np_mm),
                        "owT": owT.astype(np_mm), "mask": mask})
    return in_maps


def kernel(x, qkv_w, out_w, _trace=False, _trace_cores=None):
    nc = _build()
    in_maps = make_in_maps(x, qkv_w, out_w)
    res = run_bass_kernel_spmd(
        nc, in_maps, core_ids=list(range(NCORES)),
        trace=_trace, trace_cores=_trace_cores,
    )
    outs = [r["out"] for r in res.results]
    full = np.stack([
        outs[0] + outs[1] + outs[2] + outs[3],
        outs[4] + outs[5] + outs[6] + outs[7],
    ]).astype(np.float32)
    if _trace:
        return full, res
    return full


# revision 12
# speedup vs baseline: 1.1711x; 1.1711x over previous
"""Causal multi-head attention block (qkv proj + causal softmax attention + out proj)
for Trainium2, sharded over 8 NeuronCores: data-parallel over batch (2) x
tensor-parallel over heads (4 heads per core of 16).

Schedule: chunk-software-pipelined. The attention inner loop (scores ->
exp -> attnT) is ACT(exp)-rate-limited, so the qkv projection of chunk
qc+1 and the out-projection of chunk qc-1 are *injected* between
attention steps via a feeder, keeping the in-order PE queue busy during
exp waits. Exps are paired: two consecutive score tiles of one head are
written offset-packed into one 2-bank PSUM tile and activated with a
single wide ACTIVATE, halving ACT instruction overhead.

Per core (batch b, 4 heads):
  qT,kT [hd, S] and v [S, hd]    (qkv projection, weights pre-transposed on host)
  ST    [k, q] = transposed scores, causal-blocked
  P     = exp(ST + mask)
  attnT [hd+1, q] = [v | 1].T @ P   (row hd = softmax denominator)
  attn_n = attnT / denom
  out_partial [S, D] = attn_n.T @ owT  (row-parallel out proj)
Host sums the 4 per-core partials of each batch.
"""

import os
import sys

import numpy as np

sys.path.insert(0, "/opt/trn_rl_repo")

import concourse.bass as bass
import concourse.tile as tile
from concourse import bacc, mybir
from concourse.bass import MemorySpace
from concourse.bass_utils import run_bass_kernel_spmd

F32 = mybir.dt.float32
EXP = mybir.ActivationFunctionType.Exp
BF16 = mybir.dt.bfloat16

B, S, D = 2, 2048, 1024
H, HD = 16, 64
NCORES = 8
NH = 4          # heads per core
NP = 2          # head pairs per core
SCALE = HD ** -0.5

N_DT = D // 128          # 8 d-tiles of 128
N_ST = S // 128          # 16 seq tiles of 128
N_CH = S // 512          # 4 seq chunks of 512
FQK = 2 * NH * HD // 128  # 4 f-tiles covering q|k (pair-major)
VW = NH * HD             # 256 v columns
NEG = -1.0e9

MM_DT = BF16
DEPTH = 1   # attnT pipeline depth, in kt-PAIR units (= 2 kt of slack)


def _emit(tc, nc, xT_d, wT_d, owT_d, mask_d, out_d):
    import contextlib

    ctx = contextlib.ExitStack()
    with ctx:
        # ---------------- pools ----------------
        sb = ctx.enter_context(tc.tile_pool(name="sb", bufs=1))
        p_pool = ctx.enter_context(tc.tile_pool(name="psb", bufs=6))
        an_pool = ctx.enter_context(tc.tile_pool(name="attn_n", bufs=4))
        sm_pool = ctx.enter_context(tc.tile_pool(name="smalls", bufs=8))
        out_pool = ctx.enter_context(tc.tile_pool(name="outsb", bufs=4))
        # PSUM: 8 banks of [128 x 2KB]:
        #   ps_st : 2 x [128,1024] f32 (paired score tiles)      -> 4 banks
        #   ps_at : 2 x [65->128,512] f32 (attnT accumulators)   -> 2 banks
        #   ps_acc: 2 x [128,512] f32 (qkv + outproj chains)     -> 2 banks
        ps_acc = ctx.enter_context(
            tc.tile_pool(name="ps_acc", bufs=2, space=MemorySpace.PSUM))
        ps_at = ctx.enter_context(
            tc.tile_pool(name="ps_at", bufs=2, space=MemorySpace.PSUM))
        ps_st = ctx.enter_context(
            tc.tile_pool(name="ps_st", bufs=2, space=MemorySpace.PSUM))

        # ---------------- static SBUF tiles ----------------
        # QK (pair-major): tile f in {0,1} = qT of heads 2f/2f+1 in rows
        # 0:64/64:128; tile 2+f = kT likewise. Scores run K=64 with both
        # operands at base partition 64*(h%2).
        qk_sb = [sb.tile([128, S], MM_DT, tag=f"qk{i}", name=f"qk{i}")
                 for i in range(NH)]
        # v: per seq-tile [128, 4 heads, 65] (64 v cols + ones col)
        v_sb = [sb.tile([128, NH, HD + 1], MM_DT, tag=f"v{i}", name=f"v{i}")
                for i in range(N_ST)]
        mask_sb = sb.tile([128, 640], F32)
        owT_sb = [sb.tile([128, D], MM_DT, tag=f"ow{i}", name=f"ow{i}")
                  for i in range(NP)]
        xT_sb = [sb.tile([128, S], MM_DT, tag=f"x{i}", name=f"x{i}")
                 for i in range(N_DT)]
        wT_sb = [sb.tile([128, 3 * VW], MM_DT, tag=f"w{i}", name=f"w{i}")
                 for i in range(N_DT)]
        warm_sb = sb.tile([128, 512], MM_DT)

        # init: ones columns of the v tiles (strided, cheap); runs during DMA
        nc.vector.memset(warm_sb, 0.0)
        for t in v_sb:
            nc.gpsimd.memset(t[:, :, HD:HD + 1], 1.0)

        # ---------------- input DMAs (priority order) ----------------
        # first qkv chain needs wT[d] + xT[d] cols 0:512 -> interleave those.
        nc.sync.dma_start(out=mask_sb, in_=mask_d)
        for d in range(N_DT):
            nc.sync.dma_start(out=wT_sb[d], in_=wT_d[d * 128:(d + 1) * 128, :])
            nc.sync.dma_start(out=xT_sb[d][:, 0:512],
                              in_=xT_d[d * 128:(d + 1) * 128, 0:512])
        for d in range(N_DT):
            nc.sync.dma_start(
                out=xT_sb[d][:, 512:1024],
                in_=xT_d[d * 128:(d + 1) * 128, 512:1024])
        for p in range(NP):
            nc.sync.dma_start(out=owT_sb[p], in_=owT_d[p * 128:(p + 1) * 128, :])
        for sch in range(2, N_CH):
            for d in range(N_DT):
                nc.sync.dma_start(
                    out=xT_sb[d][:, sch * 512:(sch + 1) * 512],
                    in_=xT_d[d * 128:(d + 1) * 128, sch * 512:(sch + 1) * 512])

        # HAM warm-up: dependency-free matmuls ramp the PE clock to 8/8
        # while the input DMAs stream.
        wu_ps = ps_st.tile([128, 1024], F32, tag="st", name="wu_ps")
        for _ in range(30):
            nc.tensor.matmul(wu_ps[:, 0:512], warm_sb[:, 0:128], warm_sb,
                             start=True, stop=True)

        # ---------------- feeder item generators ----------------
        def chain_qk(f, sch):
            """qT/kT projection chain: out[f 128, s 512] += wT.T @ xT.

            f in {0,1} -> qT of heads 2f/2f+1; f in {2,3} -> kT of heads
            2(f-2)/2(f-2)+1. One full [128,512] copy into qk_sb[f]."""
            pss = ps_acc.tile([128, 512], F32, tag="acc", name="psqk")
            for d in range(N_DT):
                yield lambda d=d, pss=pss: nc.tensor.matmul(
                    pss,
                    wT_sb[d][:, f * 128:(f + 1) * 128],
                    xT_sb[d][:, sch * 512:(sch + 1) * 512],
                    start=(d == 0),
                    stop=(d == N_DT - 1),
                )

            def cp(pss=pss):
                # alternate the copy between ACT and DVE per head
                if f % 2 == 0:
                    nc.scalar.copy(
                        qk_sb[f][:, sch * 512:(sch + 1) * 512], pss)
                else:
                    nc.vector.tensor_copy(
                        qk_sb[f][:, sch * 512:(sch + 1) * 512], pss)
            yield cp

        def chain_v(st):
            """v projection chain for one seq tile: [s 128, 256]."""
            psv = ps_acc.tile([128, VW], F32, tag="acc", name="psv")
            for d in range(N_DT):
                yield lambda d=d, psv=psv: nc.tensor.matmul(
                    psv,
                    xT_sb[d][:, st * 128:(st + 1) * 128],
                    wT_sb[d][:, 2 * VW:3 * VW],
                    start=(d == 0),
                    stop=(d == N_DT - 1),
                )
            yield lambda psv=psv: nc.vector.tensor_copy(
                v_sb[st][:, :, 0:HD],
                psv.rearrange("p (h d) -> p h d", h=NH),
            )

        an_hist = {}
        pend_norm = {}

        def outproj_units(qc):
            """out_partial rows qc*512:(qc+1)*512 = attn_n.T @ owT."""
            an = an_hist.pop(qc)
            k = 0
            for qs in range(4):
                qsl = slice(qs * 128, (qs + 1) * 128)
                for e in range(2):
                    ops = ps_acc.tile([128, 512], F32, tag="acc", name="psout")
                    for p in range(NP):
                        yield lambda ops=ops, p=p, an=an, qsl=qsl, e=e: \
                            nc.tensor.matmul(
                                ops,
                                an[p][:, qsl],
                                owT_sb[p][:, e * 512:(e + 1) * 512],
                                start=(p == 0),
                                stop=(p == NP - 1),
                            )

                    def cpdma(ops=ops, qs=qs, e=e, k=k):
                        osb = out_pool.tile([128, 512], F32, tag="osb",
                                            name="osb")
                        if k % 2 == 0:
                            nc.vector.tensor_copy(osb, ops)
                        else:
                            nc.scalar.copy(osb, ops)
                        nc.sync.dma_start(
                            out=out_d[qc * 512 + qs * 128:
                                      qc * 512 + (qs + 1) * 128,
                                      e * 512:(e + 1) * 512],
                            in_=osb,
                        )
                    yield cpdma
                    k += 1

        class Feeder:
            def __init__(self, gens):
                self.items = []
                for g in gens:
                    self.items.append(g)
                self.emitted = 0
                self.cur = None
                self.total = 0

            def pump_frac(self, frac):
                tgt = int(self.total * frac + 0.999)
                while self.emitted < tgt:
                    if self.cur is None:
                        if not self.items:
                            return
                        self.cur = self.items.pop(0)
                    try:
                        thunk = next(self.cur)
                    except StopIteration:
                        self.cur = None
                        continue
                    thunk()
                    self.emitted += 1

        # ---------------- attention chunk ----------------
        def emit_attention(qc, feeder):
            n_kt = 4 * (qc + 1)
            n_pairs = n_kt // 2
            steps_per_p = n_pairs + DEPTH
            tot_steps = NP * steps_per_p
            step = 0
            for p in range(NP):
                at_ps = [ps_at.tile([HD + 1, 512], F32, tag="at", name="at_ps")
                         for _ in range(2)]
                pend = {}
                for m in range(steps_per_p):
                    if m < n_pairs:
                        kt0, kt1 = 2 * m, 2 * m + 1
                        j0, j1 = kt0 - 4 * qc, kt1 - 4 * qc
                        rs0 = 0 if j0 < 0 else j0 * 128
                        rs1 = 0 if j1 < 0 else j1 * 128
                        n0, n1 = 512 - rs0, 512 - rs1
                        for hp in range(2):
                            h = 2 * p + hp
                            qt, kt_, rb = qk_sb[p], qk_sb[2 + p], hp * HD
                            st_t = ps_st.tile([128, 1024], F32, tag="st",
                                              name="st_t")
                            # kt0 scores at cols [0:n0] (K=64, same base
                            # partition rb for both operands)
                            nc.tensor.matmul(
                                st_t[:, 0:n0],
                                kt_[rb:rb + HD, kt0 * 128:(kt0 + 1) * 128],
                                qt[rb:rb + HD, qc * 512 + rs0:(qc + 1) * 512],
                                start=True, stop=True,
                            )
                            if j0 >= 0:
                                nc.vector.tensor_add(
                                    st_t[:, 0:128], st_t[:, 0:128],
                                    mask_sb[:, 128:256])
                            # kt1 scores packed at cols [n0:n0+n1]
                            nc.tensor.matmul(
                                st_t[:, n0:n0 + n1],
                                kt_[rb:rb + HD, kt1 * 128:(kt1 + 1) * 128],
                                qt[rb:rb + HD, qc * 512 + rs1:(qc + 1) * 512],
                                start=True, stop=True,
                            )
                            if j1 >= 0:
                                nc.vector.tensor_add(
                                    st_t[:, n0:n0 + 128], st_t[:, n0:n0 + 128],
                                    mask_sb[:, 128:256])
                            # one wide exp over both tiles
                            p_t = p_pool.tile([128, 1024], MM_DT, tag="p",
                                              name="p_t")
                            nc.scalar.activation(
                                p_t[:, 0:n0 + n1], st_t[:, 0:n0 + n1], EXP)
                            pend[(m, hp)] = (p_t, rs0, n0, rs1, n1)
                    if m >= DEPTH:
                        md = m - DEPTH
                        for hp in range(2):
                            p_t, rs0, n0, rs1, n1 = pend.pop((md, hp))
                            kt0, kt1 = 2 * md, 2 * md + 1
                            nc.tensor.matmul(
                                at_ps[hp][:, rs0:512],
                                v_sb[kt0][:, 2 * p + hp, :],
                                p_t[:, 0:n0],
                                start=(md == 0),
                                stop=False,
                            )
                            nc.tensor.matmul(
                                at_ps[hp][:, rs1:512],
                                v_sb[kt1][:, 2 * p + hp, :],
                                p_t[:, n0:n0 + n1],
                                start=False,
                                stop=(md == n_pairs - 1),
                            )
                    step += 1
                    feeder.pump_frac(step / tot_steps)
                # release the accumulator banks fast (two DVE copies);
                # the reciprocal chain is deferred to emit_normalize.
                for hp in range(2):
                    anu = an_pool.tile([HD, 512], MM_DT, tag="anu", name="anu")
                    nc.vector.tensor_copy(anu, at_ps[hp][0:HD, :])
                    lsb = sm_pool.tile([1, 512], F32, tag="lsb", name="lsb")
                    nc.vector.tensor_copy(lsb, at_ps[hp][HD:HD + 1, :])
                    pend_norm[(qc, p, hp)] = (anu, lsb)
            feeder.pump_frac(1.0)

        def norm_gen(qc):
            """Normalize attnT by the softmax denominator.

            Yielded as feeder items with bcast(i+1) emitted before mul(i) so
            a DVE mul never head-blocks waiting on the gpsimd broadcast."""
            an = [an_pool.tile([128, 512], MM_DT, tag=f"an{p}", name=f"an{p}",
                               bufs=4)
                  for p in range(NP)]
            an_hist[qc] = an
            units = [(p, hp) for p in range(NP) for hp in range(2)]
            recs = []

            def recs_thunk():
                for p, hp in units:
                    anu, lsb = pend_norm.pop((qc, p, hp))
                    rec = sm_pool.tile([1, 512], F32, tag="rec", name="rec",
                                       bufs=2)
                    nc.vector.reciprocal_approx_fast(rec, lsb)
                    rec16 = sm_pool.tile([1, 512], MM_DT, tag="rec16",
                                         name="rec16", bufs=8)
                    nc.vector.tensor_copy(rec16, rec)
                    recs.append((p, hp, anu, rec16))
            yield recs_thunk

            bcs = []

            def bc_thunk(i):
                p, hp, anu, rec16 = recs[i]
                bc = sm_pool.tile([HD, 512], MM_DT, tag="bc", name="bc",
                                  bufs=4)
                nc.gpsimd.partition_broadcast(bc, rec16)
                bcs.append(bc)

            def mul_thunk(i):
                p, hp, anu, rec16 = recs[i]
                nc.vector.tensor_mul(
                    an[p][hp * HD:(hp + 1) * HD, :], anu, bcs[i])

            yield lambda: bc_thunk(0)
            yield lambda: bc_thunk(1)
            yield lambda: mul_thunk(0)
            yield lambda: bc_thunk(2)
            yield lambda: mul_thunk(1)
            yield lambda: bc_thunk(3)
            yield lambda: mul_thunk(2)
            yield lambda: mul_thunk(3)

        # ---------------- schedule ----------------
        # pre-loop: qk projection of chunk 0 (v(0..3) arrives via feeder)
        for it in chain_qk(0, 0):
            it()
        for it in chain_qk(1, 0):
            it()
        for it in chain_qk(2, 0):
            it()
        for it in chain_qk(3, 0):
            it()

        for qc in range(N_CH):
            gens = []
            if qc == 0:
                gens += [chain_v(st) for st in range(4)]
                gens += [chain_qk(f, 1) for f in range(FQK)]
                gens += [chain_v(st) for st in range(4, 8)]
                total = 9 * 12
            elif qc < N_CH - 1:
                gens += [norm_gen(qc - 1)]
                gens += [chain_qk(f, qc + 1) for f in range(FQK)]
                gens += [chain_v(st) for st in range(4 * qc + 4, 4 * qc + 8)]
                gens += [outproj_units(qc - 1)]
                total = 9 + 9 * 8 + 32
            else:
                gens = [norm_gen(qc - 1), outproj_units(qc - 1)]
                total = 9 + 32
            feeder = Feeder(gens)
            feeder.total = total
            emit_attention(qc, feeder)

        for it in norm_gen(N_CH - 1):
            it()
        for it in outproj_units(N_CH - 1):
            it()


_CACHE = {}


def _build():
    if "nc" in _CACHE:
        return _CACHE["nc"]
    nc = bacc.Bacc("TRN2", target_bir_lowering=False, debug=False)
    xT_d = nc.dram_tensor("xT", [D, S], MM_DT, kind="ExternalInput").ap()
    wT_d = nc.dram_tensor("wT", [D, 3 * VW], MM_DT, kind="ExternalInput").ap()
    owT_d = nc.dram_tensor("owT", [VW, D], MM_DT, kind="ExternalInput").ap()
    mask_d = nc.dram_tensor("mask", [128, 640], F32, kind="ExternalInput").ap()
    out_d = nc.dram_tensor("out", [S, D], F32, kind="ExternalOutput").ap()
    with tile.TileContext(nc) as tc:
        _emit(tc, nc, xT_d, wT_d, owT_d, mask_d, out_d)
    nc.compile()
    _CACHE["nc"] = nc
    return nc


def _mask_np():
    # [128, 640]: cols 0-127 all NEG, cols 128-255 lower-triangular keep
    # (col >= row -> 0 else NEG), cols 256-639 zeros.
    m = np.zeros((128, 640), np.float32)
    m[:, 0:128] = NEG
    r = np.arange(128)
    tri = np.where(r[None, :] >= r[:, None], 0.0, NEG).astype(np.float32)
    m[:, 128:256] = tri
    return m


def make_in_maps(x, qkv_w, out_w):
    """Per-core input dicts for the 8-way (batch x head-group) sharding."""
    x = np.asarray(x, np.float32)
    qkv_w = np.asarray(qkv_w, np.float32)
    out_w = np.asarray(out_w, np.float32)
    xT = [np.ascontiguousarray(x[b].T) for b in range(B)]
    mask = _mask_np()
    import ml_dtypes
    np_mm = ml_dtypes.bfloat16
    in_maps = []
    for c in range(NCORES):
        b = c // 4
        h0 = (c % 4) * NH
        rows = np.arange(h0 * HD, (h0 + NH) * HD)
        wq = qkv_w[rows] * np.float32(SCALE)
        wk = qkv_w[D + rows]
        wv = qkv_w[2 * D + rows]
        wT = np.ascontiguousarray(np.concatenate([wq, wk, wv], 0).T)
        owT = np.ascontiguousarray(out_w[:, rows].T)
        in_maps.append({"xT": xT[b].astype(np_mm), "wT": wT.astype(np_mm),
                        "owT": owT.astype(np_mm), "mask": mask})
    return in_maps


def kernel(x, qkv_w, out_w, _trace=False, _trace_cores=None):
    nc = _build()
    in_maps = make_in_maps(x, qkv_w, out_w)
    res = run_bass_kernel_spmd(
        nc, in_maps, core_ids=list(range(NCORES)),
        trace=_trace, trace_cores=_trace_cores,
    )
    outs = [r["out"] for r in res.results]
    full = np.stack([
        outs[0] + outs[1] + outs[2] + outs[3],
        outs[4] + outs[5] + outs[6] + outs[7],
    ]).astype(np.float32)
    if _trace:
        return full, res
    return full
